# revision 14
# baseline (speedup 1.0000x reference)
"""v10: straggler-derate layout. DMA engines 0 and 15 (ports of partitions
{0-3,32-35} / {92-95,124-127}) cause ~90% of severe straggler slowdowns, so
all main tiles use the partition window [4,124): engines 0/15 serve only 4
partitions per tile (half share). 6 tiles x 120 rows cover rows 0..719; the
48 remainder rows go as two 24-partition DMAs on windows [4,28) and
[100,124), spreading the extra rows over the mid engines (+8.3% each).
Per-engine rows: eng0/15: 24, eng1/14: 48, others: 52 (vs uniform 48).
A 20-25% slow engine 0/15 now finishes far inside the conveyor instead of
extending it by 8-15 us.

SBUF: tin 6x16 KiB + tout 4x16 KiB (ring of 4, reuse gated on per-ring store
sems) + remainder in/out 16 KiB each = 192 KiB/partition.
"""

import numpy as np

_B, _C, _H, _W = 32, 3, 512, 512
_N_CORES = 8
_ROWS = (_B // _N_CORES) * _C * (_H // 8)  # 768
_COLS = 8 * _W                             # 4096
_TROWS = 120                               # main-tile rows (partitions 4..123)
_N_TILES = 6                               # 6*120 = 720 rows
_REM = _ROWS - _N_TILES * _TROWS           # 48 remainder rows
_OUT_BUFS = 4

_nc_cache = None


def _build():
    import concourse.mybir as mybir
    from concourse import bacc

    nc = bacc.Bacc(
        "TRN2", target_bir_lowering=False, debug=False, num_devices=_N_CORES
    )
    x = nc.dram_tensor(
        "x", (_ROWS, _COLS), mybir.dt.float32, kind="ExternalInput"
    ).ap()
    y = nc.dram_tensor(
        "y", (_ROWS, _COLS), mybir.dt.float32, kind="ExternalOutput"
    ).ap()

    f32 = mybir.dt.float32
    with (
        nc.sbuf_tensor([128, _N_TILES * _COLS], f32) as tin,
        nc.sbuf_tensor([128, _OUT_BUFS * _COLS], f32) as tout,
        nc.sbuf_tensor([128, _COLS], f32) as rin,
        nc.sbuf_tensor([128, _COLS], f32) as rout,
        nc.semaphore() as sem_ld_sp,   # loads on sync (SP ring)
        nc.semaphore() as sem_ld_act,  # loads on scalar (ACT ring)
        nc.semaphore() as sem_cp,      # vector copies
        nc.semaphore() as sem_st_sp,   # stores on sync
        nc.semaphore() as sem_st_act,  # stores on scalar
    ):
        # Main loads upfront: L0,L2,L4 -> sync, L1,L3,L5 -> scalar.
        # Tile t: partitions 4..123 <- DRAM rows [120t, 120t+120).
        for t in range(_N_TILES):
            eng = nc.sync if t % 2 == 0 else nc.scalar
            sem = sem_ld_sp if t % 2 == 0 else sem_ld_act
            eng.dma_start(
                out=tin[4:124, t * _COLS:(t + 1) * _COLS],
                in_=x[_TROWS * t:_TROWS * (t + 1), :],
                single_packet=True,
            ).then_inc(sem, 16)
        # Remainder loads: rows 720..743 -> partitions 4..27 (sync, sp #4),
        # rows 744..767 -> partitions 100..123 (scalar, act #4).
        nc.sync.dma_start(
            out=rin[4:28, :], in_=x[720:744, :], single_packet=True
        ).then_inc(sem_ld_sp, 16)
        nc.scalar.dma_start(
            out=rin[100:124, :], in_=x[744:768, :], single_packet=True
        ).then_inc(sem_ld_act, 16)

        def shuffled(ap):
            return ap.rearrange("p (r bw c) -> p bw r c", r=8, bw=64, c=8)

        def shuffled_out(ap):
            return ap.rearrange("p (bw r c) -> p bw r c", bw=64, r=8, c=8)

        # Vector: 2 half-shuffles per main tile into out block t % 4.
        # Tile t >= 4 reuses the block of tile t-4: wait that store done
        # (store ring of t-4: even -> act count, odd -> sp count).
        for t in range(_N_TILES):
            sem = sem_ld_sp if t % 2 == 0 else sem_ld_act
            nc.vector.wait_ge(sem, 16 * (t // 2 + 1))
            if t >= _OUT_BUFS:
                tp = t - _OUT_BUFS
                rsem = sem_st_act if tp % 2 == 0 else sem_st_sp
                nc.vector.wait_ge(rsem, 16 * (tp // 2 + 1))
            b = t % _OUT_BUFS
            # Full 128-partition copy (compute engines need aligned partition
            # bases); partitions 0-3/124-127 shuffle junk that no store reads.
            src = shuffled(tin[:, t * _COLS:(t + 1) * _COLS])
            dst = shuffled_out(tout[:, b * _COLS:(b + 1) * _COLS])
            for s in range(2):
                bws = slice(s * 32, (s + 1) * 32)
                nc.vector.tensor_copy(out=dst[:, bws], in_=src[:, bws]).then_inc(
                    sem_cp, 1
                )
        # Remainder copy: one full-width shuffle covering both 24-partition
        # windows (aligned base requirement); waits both remainder loads.
        nc.vector.wait_ge(sem_ld_sp, 64)
        nc.vector.wait_ge(sem_ld_act, 64)
        nc.vector.tensor_copy(
            out=shuffled_out(rout[:, :]), in_=shuffled(rin[:, :])
        ).then_inc(sem_cp, 1)  # -> 13

        # Main stores: tile t (block t % 4): even -> scalar/act, odd -> sync/sp.
        for t in range(_N_TILES):
            eng = nc.scalar if t % 2 == 0 else nc.sync
            sem = sem_st_act if t % 2 == 0 else sem_st_sp
            b = t % _OUT_BUFS
            eng.wait_ge(sem_cp, 2 * t + 2)
            eng.dma_start(
                out=y[_TROWS * t:_TROWS * (t + 1), :],
                in_=tout[4:124, b * _COLS:(b + 1) * _COLS],
                single_packet=True,
            ).then_inc(sem, 16)
        # Remainder stores: RA -> scalar (act #4), RB -> sync (sp #4).
        nc.scalar.wait_ge(sem_cp, 13)  # remainder copy done
        nc.scalar.dma_start(
            out=y[720:744, :], in_=rout[4:28, :], single_packet=True
        ).then_inc(sem_st_act, 16)
        nc.sync.wait_ge(sem_cp, 13)
        nc.sync.dma_start(
            out=y[744:768, :], in_=rout[100:124, :], single_packet=True
        ).then_inc(sem_st_sp, 16)

        # Completion gate: 4 stores per ring.
        nc.gpsimd.wait_ge(sem_st_act, 64)
        nc.gpsimd.wait_ge(sem_st_sp, 64)

        nc.compile()
    return nc


def kernel(x: np.ndarray) -> np.ndarray:
    from concourse import bass_utils

    global _nc_cache
    if _nc_cache is None:
        _nc_cache = _build()
    nc = _nc_cache

    x = np.ascontiguousarray(x, dtype=np.float32)
    assert x.shape == (_B, _C, _H, _W), x.shape
    xs = x.reshape(_N_CORES, _ROWS, _COLS)
    in_maps = [{"x": xs[k]} for k in range(_N_CORES)]
    res = bass_utils.run_bass_kernel_spmd(
        nc, in_maps, core_ids=list(range(_N_CORES))
    )
    ys = np.stack([res.results[k]["y"] for k in range(_N_CORES)], axis=0)
    return ys.reshape(_B, _C, 1, _H, _W)


# revision 16
# speedup vs baseline: 1.4292x; 1.4292x over previous
"""Trainium2 Bass kernel for nn_DCTLayer: 8x8 block DCT-II followed by its exact
inverse (torch_dct norm=None convention). The DCT->IDCT round trip is the
identity map in exact arithmetic, so the layer reduces to the block-layout
permutation (B, C, H, W) -> (B, C, 1, H, W) where out[b, c, 0] is the row-major
flatten of the (H/8, W/8, 8, 8) block view of the input. Computing the
permutation exactly is strictly more accurate than the reference's own fp32 FFT
round trip (rel err ~1e-7 against it).

Distribution (pure data parallelism over batch, 8 cores, no communication):
  - core k handles batches 4k..4k+4 = 12 images of 512x512 f32 (12 MiB).
  - Input viewed as [768, 4096]: each row chunk = 8 consecutive image rows
    (16 KiB, DRAM-contiguous) -> one SBUF partition.
  - On-chip shuffle per partition (vector engine, 4D access patterns):
    free-dim permutation (r, bw, c) -> (bw, r, c) with r=8, bw=64, c=8.
  - Output [768, 4096] is DRAM-contiguous per partition too, so both DMAs run
    at full descriptor efficiency. Binding resource: per-engine SDMA port rate
    (~26.5 GB/s x 16 engines) -> ~60 us conveyor per core.

Raw bass (no TileContext), minimal semaphore protocol:
  - all 6 tile loads issued up front, alternating the two HWDGE rings
    (SP=sync, ACT=scalar) so the DMA engines fill from both descriptor
    generators at once; the whole 96 KiB/partition input stays resident.
  - 2 half-tile shuffles per tile on vector, each into a dedicated out
    buffer column (96 KiB/partition out, no buffer reuse -> no waits).
  - stores split per half-tile across both rings behind the copies.
  - exactly one completion wait (gpsimd, single store sem) gates the NEFF
    epilogue; the wrapper re-clears all kernel sems at each execution's
    preamble, so no kernel-side sem cleanup is emitted at all.
"""

import numpy as np

_B, _C, _H, _W = 32, 3, 512, 512
_N_CORES = 8
_ROWS = (_B // _N_CORES) * _C * (_H // 8)  # 768 row chunks per core
_COLS = 8 * _W                             # 4096 f32 per chunk
_N_TILES = _ROWS // 128                    # 6 tiles of [128, 4096]

_nc_cache = None


def _build():
    import concourse.mybir as mybir
    from concourse import bacc

    nc = bacc.Bacc(
        "TRN2", target_bir_lowering=False, debug=False, num_devices=_N_CORES
    )
    x = nc.dram_tensor(
        "x", (_ROWS, _COLS), mybir.dt.float32, kind="ExternalInput"
    ).ap()
    y = nc.dram_tensor(
        "y", (_ROWS, _COLS), mybir.dt.float32, kind="ExternalOutput"
    ).ap()

    f32 = mybir.dt.float32
    with (
        nc.sbuf_tensor([128, _N_TILES * _COLS], f32) as tin,
        nc.sbuf_tensor([128, _N_TILES * _COLS], f32) as tout,
        nc.semaphore() as sem_ld_sp,   # loads on sync (SP ring)
        nc.semaphore() as sem_ld_act,  # loads on scalar (ACT ring)
        nc.semaphore() as sem_cp,      # vector copies
        nc.semaphore() as sem_st,      # stores, both rings
    ):
        # All 6 loads issued up front: L0,L2,L4 -> sync, L1,L3,L5 -> scalar.
        # tin column block t holds DRAM rows [128t, 128t+128).
        for t in range(_N_TILES):
            eng = nc.sync if t % 2 == 0 else nc.scalar
            sem = sem_ld_sp if t % 2 == 0 else sem_ld_act
            eng.dma_start(
                out=tin[:, t * _COLS:(t + 1) * _COLS],
                in_=x[t * 128:(t + 1) * 128, :],
                single_packet=True,
            ).then_inc(sem, 16)

        # Vector: per tile, 2 half-shuffles (bw split) into a dedicated out
        # buffer column (no reuse).
        for t in range(_N_TILES):
            sem = sem_ld_sp if t % 2 == 0 else sem_ld_act
            nc.vector.wait_ge(sem, 16 * (t // 2 + 1))
            src = tin[:, t * _COLS:(t + 1) * _COLS].rearrange(
                "p (r bw c) -> p bw r c", r=8, bw=64, c=8
            )
            dst = tout[:, t * _COLS:(t + 1) * _COLS].rearrange(
                "p (bw r c) -> p bw r c", bw=64, r=8, c=8
            )
            for s in range(2):
                bws = slice(s * 32, (s + 1) * 32)
                nc.vector.tensor_copy(out=dst[:, bws], in_=src[:, bws]).then_inc(
                    sem_cp, 1
                )

        # Stores: full-tile per tile, except the FIRST and LAST tiles are
        # column-split in two (copy half s produces output cols
        # [2048s, 2048s+2048)): the first half-store enters the conveyor one
        # copy-half earlier, and the last tile's first half enqueues before
        # the final copy finishes, trimming the end drain. All transfers
        # keep the full 128-partition width. Per-ring waits stay ascending.
        def store(eng, rows, cols, cp_need):
            eng.wait_ge(sem_cp, cp_need)
            eng.dma_start(
                out=y[rows, cols],
                in_=tout[:, cols.start + rows.start // 128 * _COLS:
                         cols.stop + rows.start // 128 * _COLS],
                single_packet=True,
            ).then_inc(sem_st, 16)

        half0, half1, full = slice(0, 2048), slice(2048, 4096), slice(0, _COLS)
        store(nc.scalar, slice(0, 128), half0, 1)
        store(nc.sync, slice(0, 128), half1, 2)
        store(nc.sync, slice(128, 256), full, 4)
        store(nc.scalar, slice(256, 384), full, 6)
        store(nc.sync, slice(384, 512), full, 8)
        store(nc.scalar, slice(512, 640), full, 10)
        store(nc.sync, slice(640, 768), half0, 11)
        store(nc.scalar, slice(640, 768), half1, 12)

        # Single completion gate: 8 stores counted on one sem. The NEFF
        # preamble re-clears the kernel sem range every execution, so no
        # kernel-side sem cleanup is needed.
        nc.gpsimd.wait_ge(sem_st, 16 * 8)

        nc.compile()
    return nc


def kernel(x: np.ndarray) -> np.ndarray:
    from concourse import bass_utils

    global _nc_cache
    if _nc_cache is None:
        _nc_cache = _build()
    nc = _nc_cache

    x = np.ascontiguousarray(x, dtype=np.float32)
    assert x.shape == (_B, _C, _H, _W), x.shape
    xs = x.reshape(_N_CORES, _ROWS, _COLS)
    in_maps = [{"x": xs[k]} for k in range(_N_CORES)]
    res = bass_utils.run_bass_kernel_spmd(
        nc, in_maps, core_ids=list(range(_N_CORES))
    )
    ys = np.stack([res.results[k]["y"] for k in range(_N_CORES)], axis=0)
    return ys.reshape(_B, _C, 1, _H, _W)
